# revision 13
# baseline (speedup 1.0000x reference)
"""Trainium2 Bass kernel for nn_ConLoss (supervised-contrastive loss).

Strategy (data-parallel over batch, B=8 == n_cores):
  - Each core handles one batch b. Device computes, per (l, row i):
        S[l, i] = sum_j neg_mask[i, j] * exp(cos(w_li, w_lj) / TEMP)
    via: row-normalize w -> PE transpose -> G = wn @ wn^T in PSUM (fp32r),
    with the negative-pair mask folded in additively through a one-hot
    (K=41) matmul accumulated into the same PSUM bank (masked entries get
    a -120 logit shift => exp underflows to exactly 0), then one fused
    ScalarE pass exp + row-sum (accum_out).
  - Host replicates the reference's discrete logic (random positive
    selection, last-negative index, padding, gating) on the tiny int
    tensors and assembles the scalar loss in float64.
"""

import numpy as np

B, N, T, L, D = 8, 128, 8, 6, 256
P = N * T  # 1024
TEMP = 0.07
NCLASS = 40
MM = 8.4  # additive mask magnitude in cosine units -> 120 in logit units
NCORES = 8
NIT = P // 128  # 8 row-tiles per core

_CACHE = {}


def _build_device_kernel():
    import sys

    if "/opt/trn_rl_repo" not in sys.path:
        sys.path.insert(0, "/opt/trn_rl_repo")
    import concourse.bacc as bacc
    import concourse.tile as tile
    from concourse import mybir
    from concourse.masks import make_identity

    f32 = mybir.dt.float32
    bf16 = mybir.dt.bfloat16

    nc = bacc.Bacc("TRN2", target_bir_lowering=False, debug=False, num_devices=NCORES)
    w_dram = nc.dram_tensor("w", [P, L * D], f32, kind="ExternalInput")
    lhs_dram = nc.dram_tensor("lhs_aug", [NCLASS + 1, P], bf16, kind="ExternalInput")
    rhs_dram = nc.dram_tensor("rhs_aug", [NCLASS + 1, P], bf16, kind="ExternalInput")
    s_dram = nc.dram_tensor("s", [128, L * NIT], f32, kind="ExternalOutput")

    with tile.TileContext(nc) as tc:
        with (
            tc.tile_pool(name="singles", bufs=1) as singles,
            tc.tile_pool(name="wload", bufs=2) as wload,
            tc.tile_pool(name="wn", bufs=2) as wnp,
            tc.tile_pool(name="wnt", bufs=2) as wntp,
            tc.tile_pool(name="small", bufs=4) as small,
            tc.tile_pool(name="escr", bufs=3) as escrp,
            tc.tile_pool(name="zpsum", bufs=4, space="PSUM") as zpsum,
        ):
            lhs_sb = singles.tile([NCLASS + 1, P], bf16)
            nc.sync.dma_start(out=lhs_sb, in_=lhs_dram[:, :])
            rhs_sb = singles.tile([NCLASS + 1, P], bf16)
            nc.sync.dma_start(out=rhs_sb, in_=rhs_dram[:, :])
            stage = singles.tile([128, L * NIT], f32)

            # w viewed as [p-tile, it, l, d] for one-DMA-per-layer loads
            w_view = w_dram.ap().rearrange(
                "(it p) (l d) -> p it l d", p=128, d=D
            )  # [128, 8, 6, 256]

            for l in range(L):
                wnT = [
                    wntp.tile([128, P], bf16, tag=f"wnt{dc}", name=f"wnt{dc}_{l}")
                    for dc in range(2)
                ]
                # one batched load: all 8 row-tiles of layer l
                wl = wload.tile([128, NIT, D], f32, name=f"wl_{l}")
                nc.sync.dma_start(out=wl, in_=w_view[:, :, l, :])
                # row sums of squares for all 8 tiles: [128, 8]
                sq = wnp.tile([128, NIT, D], f32, tag="sq", name=f"sq_{l}")
                nc.vector.tensor_mul(sq, wl, wl)
                ssq = small.tile([128, NIT], f32, tag="ssq", name=f"ssq_{l}")
                nc.vector.tensor_reduce(
                    ssq, sq, axis=mybir.AxisListType.X, op=mybir.AluOpType.add
                )
                ssq2 = small.tile([128, NIT], f32, tag="ssq2", name=f"ssq2_{l}")
                nc.vector.tensor_scalar_max(ssq2, ssq, 1e-24)
                # rinv = exp(-0.5 * ln(ssq)); Ln and Exp share one ACT table set
                lssq = small.tile([128, NIT], f32, tag="lssq", name=f"lssq_{l}")
                nc.scalar.activation(lssq, ssq2, mybir.ActivationFunctionType.Ln)
                rinv = small.tile([128, NIT], f32, tag="rinv", name=f"rinv_{l}")
                nc.scalar.activation(
                    rinv, lssq, mybir.ActivationFunctionType.Exp, scale=-0.5
                )
                wn = wnp.tile([128, NIT, D], bf16, tag="wn", name=f"wn_{l}")
                for it in range(NIT):
                    nc.gpsimd.tensor_scalar_mul(
                        wn[:, it, :], wl[:, it, :], rinv[:, it : it + 1]
                    )
                    for dc in range(2):
                        nc.sync.dma_start(
                            out=wnT[dc][:, it * 128 : (it + 1) * 128],
                            in_=wn[:, it, dc * 128 : (dc + 1) * 128],
                            transpose=True,
                        )
                for it in range(NIT):
                    pz = zpsum.tile([128, P], f32)
                    isl = slice(it * 128, (it + 1) * 128)
                    for nh in range(2):
                        nsl = slice(nh * 512, (nh + 1) * 512)
                        nc.tensor.matmul(
                            pz[:, nsl],
                            lhsT=lhs_sb[:, isl],
                            rhs=rhs_sb[:, nsl],
                            start=True,
                            stop=False,
                        )
                        for dc in range(2):
                            nc.tensor.matmul(
                                pz[:, nsl],
                                lhsT=wnT[dc][:, isl],
                                rhs=wnT[dc][:, nsl],
                                start=False,
                                stop=(dc == 1),
                            )
                    escr = escrp.tile([128, P], f32)
                    nc.scalar.activation(
                        escr,
                        pz,
                        mybir.ActivationFunctionType.Exp,
                        scale=1.0 / TEMP,
                        accum_out=stage[:, l * NIT + it : l * NIT + it + 1],
                    )

            nc.sync.dma_start(out=s_dram[:, :], in_=stage)

    nc.compile()
    return nc


def _get_nc():
    if "nc" not in _CACHE:
        _CACHE["nc"] = _build_device_kernel()
    return _CACHE["nc"]


def _device_masked_expsums(meta, lab, valid, trace=False):
    """Run the Bass kernel on 8 cores. Returns S[b, l, i] = masked sum of
    exp(z) over negatives, plus the BassKernelResults for profiling."""
    import sys

    if "/opt/trn_rl_repo" not in sys.path:
        sys.path.insert(0, "/opt/trn_rl_repo")
    from concourse.bass_utils import run_bass_kernel_spmd

    import ml_dtypes

    nc = _get_nc()
    bf16 = ml_dtypes.bfloat16
    cls = np.arange(NCLASS, dtype=np.int32)
    in_maps = []
    for b in range(B):
        w_b = np.ascontiguousarray(meta[b].reshape(P, L * D))
        onehot = (lab[b][None, :] == cls[:, None]).astype(np.float32)  # [40, P]
        vj = valid[b].astype(np.float32)  # [P]
        lhs_aug = np.concatenate([onehot, np.ones((1, P), np.float32)], axis=0)
        rhs_aug = np.concatenate(
            [-MM * vj[None, :] * onehot, (MM * (vj - 1.0))[None, :]], axis=0
        )
        in_maps.append(
            {
                "w": w_b,
                "lhs_aug": lhs_aug.astype(bf16),
                "rhs_aug": rhs_aug.astype(bf16),
            }
        )

    res = run_bass_kernel_spmd(nc, in_maps, list(range(NCORES)), trace=trace)
    # s[p, l*8+it] is the row-sum for row i = it*128 + p of layer l
    S = np.stack([res.results[c]["s"] for c in range(NCORES)])  # [B, 128, L*8]
    S = S.reshape(B, 128, L, NIT).transpose(0, 2, 3, 1).reshape(B, L, P)
    return S, res


def _host_assemble(meta, thing, lab, valid, S_dev):
    """Replicate the reference's discrete selection logic and assemble the
    final scalar loss (float64 assembly; discrete index math bit-exact)."""
    import jax

    same = lab[:, :, None] == lab[:, None, :]  # [B,P,P]
    vpair = valid[:, :, None] & valid[:, None, :]
    eye = np.eye(P, dtype=bool)
    pos_mask = same & ~eye[None] & vpair
    neg_mask = ~same & vpair

    K = valid.sum(axis=1).astype(np.int64)  # [B]
    pos_count = pos_mask.sum(axis=-1).astype(np.int32)  # [B,P]
    neg_count = neg_mask.sum(axis=-1).astype(np.int64)  # [B,P]

    u = np.asarray(jax.random.uniform(jax.random.key(42), (B, L, P)))  # f32
    r = np.floor(u * pos_count[:, None, :].astype(np.float32)).astype(np.int32)
    r = np.minimum(r, np.maximum(pos_count - 1, 0)[:, None, :].astype(np.int32))
    ranks = np.cumsum(pos_mask, axis=-1).astype(np.int32) - 1  # [B,P,P]
    sel = pos_mask[:, None, :, :] & (ranks[:, None, :, :] == r[..., None])
    jsel = sel.argmax(axis=-1)  # [B,L,P]
    jsel = np.where(pos_count[:, None, :] > 0, jsel, np.arange(P)[None, None, :])

    iota = np.arange(P)
    idx_last = np.max(np.where(neg_mask, iota[None, None, :], -1), axis=-1)  # [B,P]
    jlast = np.maximum(idx_last, 0)

    # normalized weights (f32, matching reference's normalize)
    wf = meta.reshape(B, P, L, D)
    nrm = np.maximum(np.linalg.norm(wf, axis=-1, keepdims=True), 1e-12).astype(
        np.float32
    )
    wn = (wf / nrm).transpose(0, 2, 1, 3)  # [B,L,P,D]

    wn_sel = np.take_along_axis(wn, jsel[..., None], axis=2)  # [B,L,P,D]
    zp = np.einsum("blpd,blpd->blp", wn, wn_sel, dtype=np.float64) / TEMP
    jl2 = np.broadcast_to(jlast[:, None, :], (B, L, P))
    wn_last = np.take_along_axis(wn, jl2[..., None], axis=2)
    zl = np.einsum("blpd,blpd->blp", wn, wn_last, dtype=np.float64) / TEMP

    pad = np.maximum(K[:, None] - 1 - neg_count, 0).astype(np.float64)  # [B,P]
    pad_eff = np.where(neg_count > 0, pad, 0.0)

    S = S_dev.astype(np.float64)
    loss_row = (
        np.log(np.exp(zp) + S + pad_eff[:, None, :] * np.exp(zl)) - zp
    )  # [B,L,P]
    row_w = valid.astype(np.float64)
    mean_per = (loss_row * row_w[:, None, :]).sum(axis=-1) / np.maximum(K, 1)[
        :, None
    ].astype(np.float64)
    gate = (thing.sum(axis=2) > 0).sum(axis=1) >= 2  # [B]
    total = np.where(gate[:, None], mean_per, 0.0).sum()
    return np.asarray(total, dtype=np.float32)


def kernel(meta_weight_for_con, thing_gt_idx, label_tmp):
    meta = np.asarray(meta_weight_for_con, dtype=np.float32)
    thing = np.asarray(thing_gt_idx)
    label = np.asarray(label_tmp)
    valid = thing.reshape(B, P).astype(bool)
    lab = label.reshape(B, P).astype(np.int32)

    S_dev, _ = _device_masked_expsums(meta, lab, valid)
    return _host_assemble(meta, thing, lab, valid, S_dev)


# revision 15
# speedup vs baseline: 1.6495x; 1.6495x over previous
"""Trainium2 Bass kernel for nn_ConLoss (supervised-contrastive loss).

Strategy (data-parallel over batch, B=8 == n_cores):
  - Each core handles one batch b. Device computes, per (l, row i):
        S[l, i] = sum_j neg_mask[i, j] * exp(cos(w_li, w_lj) / TEMP)
    via: row-normalize w -> PE transpose -> G = wn @ wn^T in PSUM (fp32r),
    with the negative-pair mask folded in additively through a one-hot
    (K=41) matmul accumulated into the same PSUM bank (masked entries get
    a -120 logit shift => exp underflows to exactly 0), then one fused
    ScalarE pass exp + row-sum (accum_out).
  - Host replicates the reference's discrete logic (random positive
    selection, last-negative index, padding, gating) on the tiny int
    tensors and assembles the scalar loss in float64.
"""

import numpy as np

B, N, T, L, D = 8, 128, 8, 6, 256
P = N * T  # 1024
TEMP = 0.07
NCLASS = 40
MM = 8.4  # additive mask magnitude in cosine units -> 120 in logit units
NCORES = 8
NIT = P // 128  # 8 row-tiles per core

_CACHE = {}


def _build_device_kernel():
    import sys

    if "/opt/trn_rl_repo" not in sys.path:
        sys.path.insert(0, "/opt/trn_rl_repo")
    import concourse.bacc as bacc
    import concourse.tile as tile
    from concourse import mybir
    from concourse.masks import make_identity

    f32 = mybir.dt.float32
    bf16 = mybir.dt.bfloat16

    nc = bacc.Bacc("TRN2", target_bir_lowering=False, debug=False, num_devices=NCORES)
    w_dram = nc.dram_tensor("w", [P, L * D], f32, kind="ExternalInput")
    lhs_dram = nc.dram_tensor("lhs_aug", [NCLASS + 1, P], bf16, kind="ExternalInput")
    rhs_dram = nc.dram_tensor("rhs_aug", [NCLASS + 1, P], bf16, kind="ExternalInput")
    s_dram = nc.dram_tensor("s", [128, L * NIT], f32, kind="ExternalOutput")

    with tile.TileContext(nc) as tc:
        with (
            tc.tile_pool(name="singles", bufs=1) as singles,
            tc.tile_pool(name="wload", bufs=2) as wload,
            tc.tile_pool(name="wn", bufs=2) as wnp,
            tc.tile_pool(name="wnt", bufs=2) as wntp,
            tc.tile_pool(name="small", bufs=4) as small,
            tc.tile_pool(name="escr", bufs=3) as escrp,
            tc.tile_pool(name="zpsum", bufs=3, space="PSUM") as zpsum,
            tc.tile_pool(name="tpsum", bufs=2, space="PSUM") as tpsum,
        ):
            ident = singles.tile([128, 128], bf16)
            make_identity(nc, ident)
            lhs_sb = singles.tile([NCLASS + 1, P], bf16)
            nc.sync.dma_start(out=lhs_sb, in_=lhs_dram[:, :])
            rhs_sb = singles.tile([NCLASS + 1, P], bf16)
            nc.sync.dma_start(out=rhs_sb, in_=rhs_dram[:, :])
            stage = singles.tile([128, L * NIT], f32)

            # w viewed as [p-tile, it, l, d] for one-DMA-per-layer loads
            w_view = w_dram.ap().rearrange(
                "(it p) (l d) -> p it l d", p=128, d=D
            )  # [128, 8, 6, 256]

            for l in range(L):
                wnT = [
                    wntp.tile([128, P], bf16, tag=f"wnt{dc}", name=f"wnt{dc}_{l}")
                    for dc in range(2)
                ]
                # one batched load: all 8 row-tiles of layer l
                wl = wload.tile([128, NIT, D], f32, name=f"wl_{l}")
                nc.sync.dma_start(out=wl, in_=w_view[:, :, l, :])
                # row sums of squares for all 8 tiles: [128, 8]
                sq = wnp.tile([128, NIT, D], f32, tag="sq", name=f"sq_{l}")
                nc.vector.tensor_mul(sq, wl, wl)
                ssq = small.tile([128, NIT], f32, tag="ssq", name=f"ssq_{l}")
                nc.vector.tensor_reduce(
                    ssq, sq, axis=mybir.AxisListType.X, op=mybir.AluOpType.add
                )
                ssq2 = small.tile([128, NIT], f32, tag="ssq2", name=f"ssq2_{l}")
                nc.vector.tensor_scalar_max(ssq2, ssq, 1e-24)
                # rinv = exp(-0.5 * ln(ssq)); Ln and Exp share one ACT table set
                lssq = small.tile([128, NIT], f32, tag="lssq", name=f"lssq_{l}")
                nc.scalar.activation(lssq, ssq2, mybir.ActivationFunctionType.Ln)
                rinv = small.tile([128, NIT], f32, tag="rinv", name=f"rinv_{l}")
                nc.scalar.activation(
                    rinv, lssq, mybir.ActivationFunctionType.Exp, scale=-0.5
                )
                wn = wnp.tile([128, NIT, D], bf16, tag="wn", name=f"wn_{l}")
                for it in range(NIT):
                    nc.vector.tensor_scalar_mul(
                        wn[:, it, :], wl[:, it, :], rinv[:, it : it + 1]
                    )
                    for dc in range(2):
                        pt = tpsum.tile([128, 128], bf16)
                        nc.tensor.transpose(
                            pt, wn[:, it, dc * 128 : (dc + 1) * 128], ident
                        )
                        nc.vector.tensor_copy(
                            wnT[dc][:, it * 128 : (it + 1) * 128], pt
                        )
                for it in range(NIT):
                    pz = zpsum.tile([128, P], f32)
                    isl = slice(it * 128, (it + 1) * 128)
                    for nh in range(2):
                        nsl = slice(nh * 512, (nh + 1) * 512)
                        nc.tensor.matmul(
                            pz[:, nsl],
                            lhsT=lhs_sb[:, isl],
                            rhs=rhs_sb[:, nsl],
                            start=True,
                            stop=False,
                        )
                        for dc in range(2):
                            nc.tensor.matmul(
                                pz[:, nsl],
                                lhsT=wnT[dc][:, isl],
                                rhs=wnT[dc][:, nsl],
                                start=False,
                                stop=(dc == 1),
                            )
                    escr = escrp.tile([128, P], f32)
                    nc.scalar.activation(
                        escr,
                        pz,
                        mybir.ActivationFunctionType.Exp,
                        scale=1.0 / TEMP,
                        accum_out=stage[:, l * NIT + it : l * NIT + it + 1],
                    )

            nc.sync.dma_start(out=s_dram[:, :], in_=stage)

    nc.compile()
    return nc


def _get_nc():
    if "nc" not in _CACHE:
        _CACHE["nc"] = _build_device_kernel()
    return _CACHE["nc"]


def _device_masked_expsums(meta, lab, valid, trace=False):
    """Run the Bass kernel on 8 cores. Returns S[b, l, i] = masked sum of
    exp(z) over negatives, plus the BassKernelResults for profiling."""
    import sys

    if "/opt/trn_rl_repo" not in sys.path:
        sys.path.insert(0, "/opt/trn_rl_repo")
    from concourse.bass_utils import run_bass_kernel_spmd

    import ml_dtypes

    nc = _get_nc()
    bf16 = ml_dtypes.bfloat16
    cls = np.arange(NCLASS, dtype=np.int32)
    in_maps = []
    for b in range(B):
        w_b = np.ascontiguousarray(meta[b].reshape(P, L * D))
        onehot = (lab[b][None, :] == cls[:, None]).astype(np.float32)  # [40, P]
        vj = valid[b].astype(np.float32)  # [P]
        lhs_aug = np.concatenate([onehot, np.ones((1, P), np.float32)], axis=0)
        rhs_aug = np.concatenate(
            [-MM * vj[None, :] * onehot, (MM * (vj - 1.0))[None, :]], axis=0
        )
        in_maps.append(
            {
                "w": w_b,
                "lhs_aug": lhs_aug.astype(bf16),
                "rhs_aug": rhs_aug.astype(bf16),
            }
        )

    res = run_bass_kernel_spmd(nc, in_maps, list(range(NCORES)), trace=trace)
    # s[p, l*8+it] is the row-sum for row i = it*128 + p of layer l
    S = np.stack([res.results[c]["s"] for c in range(NCORES)])  # [B, 128, L*8]
    S = S.reshape(B, 128, L, NIT).transpose(0, 2, 3, 1).reshape(B, L, P)
    return S, res


def _host_assemble(meta, thing, lab, valid, S_dev):
    """Replicate the reference's discrete selection logic and assemble the
    final scalar loss (float64 assembly; discrete index math bit-exact)."""
    import jax

    same = lab[:, :, None] == lab[:, None, :]  # [B,P,P]
    vpair = valid[:, :, None] & valid[:, None, :]
    eye = np.eye(P, dtype=bool)
    pos_mask = same & ~eye[None] & vpair
    neg_mask = ~same & vpair

    K = valid.sum(axis=1).astype(np.int64)  # [B]
    pos_count = pos_mask.sum(axis=-1).astype(np.int32)  # [B,P]
    neg_count = neg_mask.sum(axis=-1).astype(np.int64)  # [B,P]

    u = np.asarray(jax.random.uniform(jax.random.key(42), (B, L, P)))  # f32
    r = np.floor(u * pos_count[:, None, :].astype(np.float32)).astype(np.int32)
    r = np.minimum(r, np.maximum(pos_count - 1, 0)[:, None, :].astype(np.int32))
    ranks = np.cumsum(pos_mask, axis=-1).astype(np.int32) - 1  # [B,P,P]
    sel = pos_mask[:, None, :, :] & (ranks[:, None, :, :] == r[..., None])
    jsel = sel.argmax(axis=-1)  # [B,L,P]
    jsel = np.where(pos_count[:, None, :] > 0, jsel, np.arange(P)[None, None, :])

    iota = np.arange(P)
    idx_last = np.max(np.where(neg_mask, iota[None, None, :], -1), axis=-1)  # [B,P]
    jlast = np.maximum(idx_last, 0)

    # normalized weights (f32, matching reference's normalize)
    wf = meta.reshape(B, P, L, D)
    nrm = np.maximum(np.linalg.norm(wf, axis=-1, keepdims=True), 1e-12).astype(
        np.float32
    )
    wn = (wf / nrm).transpose(0, 2, 1, 3)  # [B,L,P,D]

    wn_sel = np.take_along_axis(wn, jsel[..., None], axis=2)  # [B,L,P,D]
    zp = np.einsum("blpd,blpd->blp", wn, wn_sel, dtype=np.float64) / TEMP
    jl2 = np.broadcast_to(jlast[:, None, :], (B, L, P))
    wn_last = np.take_along_axis(wn, jl2[..., None], axis=2)
    zl = np.einsum("blpd,blpd->blp", wn, wn_last, dtype=np.float64) / TEMP

    pad = np.maximum(K[:, None] - 1 - neg_count, 0).astype(np.float64)  # [B,P]
    pad_eff = np.where(neg_count > 0, pad, 0.0)

    S = S_dev.astype(np.float64)
    loss_row = (
        np.log(np.exp(zp) + S + pad_eff[:, None, :] * np.exp(zl)) - zp
    )  # [B,L,P]
    row_w = valid.astype(np.float64)
    mean_per = (loss_row * row_w[:, None, :]).sum(axis=-1) / np.maximum(K, 1)[
        :, None
    ].astype(np.float64)
    gate = (thing.sum(axis=2) > 0).sum(axis=1) >= 2  # [B]
    total = np.where(gate[:, None], mean_per, 0.0).sum()
    return np.asarray(total, dtype=np.float32)


def kernel(meta_weight_for_con, thing_gt_idx, label_tmp):
    meta = np.asarray(meta_weight_for_con, dtype=np.float32)
    thing = np.asarray(thing_gt_idx)
    label = np.asarray(label_tmp)
    valid = thing.reshape(B, P).astype(bool)
    lab = label.reshape(B, P).astype(np.int32)

    S_dev, _ = _device_masked_expsums(meta, lab, valid)
    return _host_assemble(meta, thing, lab, valid, S_dev)
